# revision 1
# baseline (speedup 1.0000x reference)
import sys

sys.path.insert(0, "/opt/trn_rl_repo")

import numpy as np
import ml_dtypes

import concourse.bacc as bacc
import concourse.bass as bass
import concourse.mybir as mybir
import concourse.tile as tile
from concourse.bass_utils import run_bass_kernel_spmd

F32 = mybir.dt.float32
BF16 = mybir.dt.bfloat16
AF = mybir.ActivationFunctionType
ALU = mybir.AluOpType
AX = mybir.AxisListType

# Problem constants (hardcoded per harness contract).
B, C, H, W = 4, 64, 128, 128
COUT1 = 128
NT = 9          # 3x3 taps
NFF = 4         # factor*factor subpixels
NCORES = 8
HL = H // 2     # 64 coarse rows per core
NYB = 4         # y-blocks for the weighted sum
YB = HL // NYB  # 16 rows per block

_cached = {}


def ap_of(t, off, dims):
    base = t[:]
    return bass.AP(base.tensor, base.offset + off, dims)


def build_nc():
    nc = bacc.Bacc("TRN2", target_bir_lowering=False, debug=False, num_devices=NCORES)

    hp2_d = nc.dram_tensor("hp2", [128, 66 * 130], F32, kind="ExternalInput")
    h8_d = nc.dram_tensor("h8", [64, 66 * 130], BF16, kind="ExternalInput")
    w1a_d = nc.dram_tensor("w1a", [128, 3 * 128], F32, kind="ExternalInput")
    w1b_d = nc.dram_tensor("w1b", [64, 3 * 128], F32, kind="ExternalInput")
    b1_d = nc.dram_tensor("b1c", [128, 1], F32, kind="ExternalInput")
    w2t_d = nc.dram_tensor("w2t", [128, 36], F32, kind="ExternalInput")
    b2_d = nc.dram_tensor("b2c", [36, 1], F32, kind="ExternalInput")
    sel_d = nc.dram_tensor("sel", [36, 4], F32, kind="ExternalInput")
    idf_d = nc.dram_tensor("idf", [128, 128], F32, kind="ExternalInput")
    idb_d = nc.dram_tensor("idb", [128, 128], BF16, kind="ExternalInput")
    out_d = nc.dram_tensor("out", [64, H, 2 * W], F32, kind="ExternalOutput")

    NA = 4 * YB          # acc free per c: (ff, y_local)
    n = YB * 64          # per-(tap,block) product elements per partition

    with tile.TileContext(nc) as tc:
        with (
            tc.tile_pool(name="const", bufs=1) as cpool,
            tc.tile_pool(name="ring", bufs=2) as ring,
            tc.tile_pool(name="mchunk", bufs=3) as mpool,
            tc.tile_pool(name="ws1", bufs=2) as wp2,
            tc.tile_pool(name="ws2", bufs=1) as wp1,
            tc.tile_pool(name="orow", bufs=3) as opool,
            tc.tile_pool(name="ps1", bufs=2, space=bass.MemorySpace.PSUM) as pp1,
            tc.tile_pool(name="ps2", bufs=2, space=bass.MemorySpace.PSUM) as pp2,
            tc.tile_pool(name="psz", bufs=1, space=bass.MemorySpace.PSUM) as ppz,
            tc.tile_pool(name="pst", bufs=1, space=bass.MemorySpace.PSUM) as ppt,
            tc.tile_pool(name="psh", bufs=1, space=bass.MemorySpace.PSUM) as pph,
            tc.tile_pool(name="pso", bufs=1, space=bass.MemorySpace.PSUM) as ppo,
        ):
            # ---- constants ----
            w1a = cpool.tile([128, 3 * 128], F32)
            w1b = cpool.tile([64, 3 * 128], F32)
            b1 = cpool.tile([128, 1], F32)
            w2t = cpool.tile([128, 36], F32)
            b2 = cpool.tile([36, 1], F32)
            sel = cpool.tile([36, 4], F32)
            idf = cpool.tile([128, 128], F32)
            idb = cpool.tile([128, 128], BF16)
            nc.sync.dma_start(w1a[:], w1a_d[:])
            nc.sync.dma_start(w1b[:], w1b_d[:])
            nc.sync.dma_start(b1[:], b1_d[:])
            nc.sync.dma_start(w2t[:], w2t_d[:])
            nc.sync.dma_start(b2[:], b2_d[:])
            nc.sync.dma_start(sel[:], sel_d[:])
            nc.sync.dma_start(idf[:], idf_d[:])
            nc.sync.dma_start(idb[:], idb_d[:])

            for yb in range(NYB):
                r0 = yb * YB  # first coarse row of this block
                hp2b = ring.tile([128, 18 * 130], F32, tag="hp2b")
                h8b = ring.tile([64, 18 * 130], BF16, tag="h8b")
                nc.sync.dma_start(hp2b[:], hp2_d[:, r0 * 130:(r0 + 18) * 130])
                nc.sync.dma_start(h8b[:], h8_d[:, r0 * 130:(r0 + 18) * 130])

                # ---- conv1 -> relu -> conv2 -> exp -> Z -> recip (4 chunks) ----
                eb = ring.tile([36, 4 * 512], F32, tag="eb")
                rzb = ring.tile([4, 4 * 512], F32, tag="rzb")
                for ic in range(4):
                    ps1 = pp1.tile([128, 512], F32)
                    for dy in range(3):
                        rhs = ap_of(hp2b, (4 * ic + dy) * 130,
                                    [[18 * 130, 128], [130, 4], [1, 128]])
                        nc.tensor.matmul(ps1[:], w1a[:, dy * 128:(dy + 1) * 128], rhs,
                                         start=(dy == 0), stop=False)
                    for dy in range(3):
                        rhs = ap_of(hp2b, (4 * ic + dy) * 130 + 2,
                                    [[18 * 130, 64], [130, 4], [1, 128]])
                        nc.tensor.matmul(ps1[:], w1b[:, dy * 128:(dy + 1) * 128], rhs,
                                         start=False, stop=(dy == 2))
                    m = mpool.tile([128, 512], F32)
                    nc.scalar.activation(m[:], ps1[:], AF.Relu, bias=b1[:], scale=1.0)
                    ps2 = pp2.tile([40, 512], F32)
                    nc.tensor.matmul(ps2[0:36, :], w2t[:], m[:])
                    nc.scalar.activation(eb[:, ic * 512:(ic + 1) * 512],
                                         ps2[0:36, :], AF.Exp, bias=b2[:], scale=1.0)
                    psz = ppz.tile([4, 512], F32)
                    nc.tensor.matmul(psz[:], sel[:], eb[0:36, ic * 512:(ic + 1) * 512])
                    nc.vector.reciprocal(rzb[:, ic * 512:(ic + 1) * 512], psz[:])

                # ---- h transposes (bf16), batched PSUM->SBUF copies ----
                hTb = ring.tile([128, 3 * 18 * 64], BF16, tag="hTb")
                for dx in range(3):
                    for j in range(3):   # 3 batches of 6 rows
                        psh = pph.tile([128, 6 * 64], BF16)
                        for r in range(6):
                            yp = j * 6 + r
                            nc.tensor.transpose(
                                psh[:, r * 64:(r + 1) * 64],
                                ap_of(h8b, yp * 130 + dx, [[18 * 130, 64], [1, 128]]),
                                idb[0:64, 0:64])
                        nc.scalar.copy(
                            hTb[:, (dx * 18 + j * 6) * 64:(dx * 18 + j * 6 + 6) * 64],
                            psh[:])

                # ---- e/rz transposes, batched ----
                eTb = ring.tile([128, YB * 40], F32, tag="eTb")
                for j in range(4):       # 4 batches of 4 rows
                    pst = ppt.tile([128, 4 * 40], F32)
                    for r in range(4):
                        yl = j * 4 + r
                        nc.tensor.transpose(pst[:, r * 40:r * 40 + 36],
                                            eb[:, yl * 128:(yl + 1) * 128],
                                            idf[0:36, 0:36])
                        nc.tensor.transpose(pst[:, r * 40 + 36:r * 40 + 40],
                                            rzb[:, yl * 128:(yl + 1) * 128],
                                            idf[0:4, 0:4])
                    nc.scalar.copy(eTb[:, j * 160:(j + 1) * 160], pst[:])

                # ---- normalized mask, transposed+duplicated (bf16) ----
                nmb = ring.tile([128, YB * 72], BF16, tag="nmb")
                for ff in range(NFF):
                    out_ap = ap_of(nmb, ff * 18, [[YB * 72, 128], [72, YB], [2, 9], [1, 2]])
                    in0 = ap_of(eTb, ff * 9, [[YB * 40, 128], [40, YB], [1, 9], [0, 2]])
                    in1 = ap_of(eTb, 36 + ff, [[YB * 40, 128], [40, YB], [0, 9], [0, 2]])
                    nc.vector.tensor_tensor(out_ap, in0, in1, ALU.mult)

                # ---- weighted tap sum (DVE, bf16) ----
                acc = ring.tile([128, 64 * NA], F32, tag="acc")  # (c, ff, yl)
                for ff in range(NFF):
                    prod = wp2.tile([128, NT * n], BF16, tag="prod")
                    for dy in range(3):
                        for dx in range(3):
                            t = dy * 3 + dx
                            in0 = ap_of(hTb, (dx * 18 + dy) * 64,
                                        [[3 * 18 * 64, 128], [64, YB], [2, 32], [1, 2]])
                            in1 = ap_of(nmb, (ff * 9 + t) * 2,
                                        [[YB * 72, 128], [72, YB], [0, 32], [1, 2]])
                            po = ap_of(prod, t * n,
                                       [[NT * n, 128], [64, YB], [2, 32], [1, 2]])
                            nc.vector.tensor_tensor(po, in0, in1, ALU.mult)
                    tA = wp2.tile([128, 4 * n], BF16, tag="tA")
                    tB = wp1.tile([128, 2 * n], BF16, tag="tB")
                    tC = wp1.tile([128, n], BF16, tag="tC")
                    nc.vector.tensor_add(tA[:], prod[:, 0:4 * n], prod[:, 4 * n:8 * n])
                    nc.vector.tensor_add(tB[:], tA[:, 0:2 * n], tA[:, 2 * n:4 * n])
                    nc.vector.tensor_add(tC[:], tB[:, 0:n], tB[:, n:2 * n])
                    acc_ap = ap_of(acc, ff * YB, [[64 * NA, 128], [1, YB], [NA, 64]])
                    tC_ap = ap_of(tC, 0, [[n, 128], [64, YB], [1, 64]])
                    p8_ap = ap_of(prod, 8 * n, [[NT * n, 128], [64, YB], [1, 64]])
                    nc.vector.tensor_tensor(acc_ap, tC_ap, p8_ap, ALU.add)

                # ---- pixel shuffle out ----
                for yl in range(YB):
                    y = yb * YB + yl
                    orow = opool.tile([128, 256], F32)
                    for fx in range(2):
                        pso = ppo.tile([128, 128], F32)
                        in_ap = ap_of(acc, fx * YB + yl,
                                      [[64 * NA, 128], [NA, 64], [2 * YB, 2]])
                        nc.tensor.transpose(pso[:], in_ap, idf[:])
                        o_ap = ap_of(orow, fx, [[256, 128], [2, 128]])
                        nc.scalar.copy(o_ap, pso[:])
                    nc.sync.dma_start(out_d[:, 2 * y:2 * y + 2, :], orow[:])

    nc.compile()
    return nc


def prep_shared(W1, b1, W2, b2):
    W1 = np.asarray(W1, np.float32)
    b1 = np.asarray(b1, np.float32)
    W2 = np.asarray(W2, np.float32).reshape(36, 128)
    b2 = np.asarray(b2, np.float32)

    w1a = np.zeros((128, 3 * 128), np.float32)
    w1b = np.zeros((64, 3 * 128), np.float32)
    for dy in range(3):
        w1a[0:64, dy * 128:(dy + 1) * 128] = W1[:, :, dy, 0].T
        w1a[64:128, dy * 128:(dy + 1) * 128] = W1[:, :, dy, 1].T
        w1b[:, dy * 128:(dy + 1) * 128] = W1[:, :, dy, 2].T

    o_of_mp = np.array([t * 4 + ff for ff in range(4) for t in range(9)])
    w2t = np.ascontiguousarray((0.25 * W2[o_of_mp, :]).T)
    b2c = np.ascontiguousarray((0.25 * b2[o_of_mp]).reshape(36, 1))

    sel = np.zeros((36, 4), np.float32)
    for k in range(36):
        sel[k, k // 9] = 1.0
    idf = np.eye(128, dtype=np.float32)
    return {
        "w1a": w1a, "w1b": w1b, "b1c": b1.reshape(128, 1).astype(np.float32),
        "w2t": w2t.astype(np.float32), "b2c": b2c, "sel": sel, "idf": idf,
        "idb": np.eye(128, dtype=ml_dtypes.bfloat16),
    }


def kernel(h, W1, b1, W2, b2, _trace=False):
    h = np.asarray(h, np.float32)
    shared = prep_shared(W1, b1, W2, b2)

    hp = np.pad(h, ((0, 0), (0, 0), (1, 1), (1, 1)))  # [B, C, 130, 130]
    in_maps = []
    for core in range(NCORES):
        b, half = core // 2, core % 2
        y0 = half * HL
        win = hp[b, :, y0:y0 + 66, :]  # [64, 66, 130]
        hp2 = np.zeros((128, 66, 130), np.float32)
        hp2[0:64] = win
        hp2[64:128, :, 0:129] = win[:, :, 1:130]
        h8 = (8.0 * win).astype(np.float32)
        m = dict(shared)
        m["hp2"] = hp2.reshape(128, -1)
        m["h8"] = np.ascontiguousarray(h8.reshape(64, -1)).astype(ml_dtypes.bfloat16)
        in_maps.append(m)

    if "nc" not in _cached:
        _cached["nc"] = build_nc()
    res = run_bass_kernel_spmd(_cached["nc"], in_maps, core_ids=list(range(NCORES)),
                               trace=_trace)

    out = np.zeros((B, C, 2 * H, 2 * W), np.float32)
    for core in range(NCORES):
        b, half = core // 2, core % 2
        out[b, :, half * 128:(half + 1) * 128, :] = res.results[core]["out"]
    if _trace:
        return out, res
    return out

